# revision 35
# baseline (speedup 1.0000x reference)
"""GCN message-passing (GCNConv) on 8 Trainium2 NeuronCores via Bass/Tile.

Math (reference):
    deg[i] = |{e : row[e] == i}|
    h      = x @ W
    out[i] = sum_{e: row[e]==i} h[col[e]] / sqrt(deg[row[e]] * deg[col[e]])

Because the GCN norm is separable (1/sqrt(deg_i) * 1/sqrt(deg_j)), we compute
    h2   = (x @ W) * rsqrt(deg)[:, None]          (phase 1, on device)
    out  = rsqrt(deg)[:, None] * scatter_add(h2[col], row)   (phase 2)
so the per-edge work is a pure gather + segment-sum with NO per-edge flops.

Sharding: output rows are bin-packed into 8*CPC bins of <=128 rows, balanced
by edge count (the "1D row partition" of the hint, load-balanced).  Each core
owns CPC bins.  Every core computes the full h2 table (replicated XW -- it is
tiny next to the gather traffic) into its own HBM, then per bin:
  - dma_gather (GPSIMD SWDGE) pulls the bin's neighbor rows h2[col] into SBUF,
    128 edges per partition-block.  int16 gather indices only address 32k rows,
    so edges are split by col < 32768 and gathered with two table base offsets.
  - a one-hot "segment matrix" S (S[e, slot] = 1 iff edge e belongs to output
    slot) is built on the vector engine with a broadcast is_equal, and the
    segment-sum runs as PE matmuls accumulating into PSUM: out = S.T @ msgs.
  - PSUM is evacuated through a fused *rsqrt(deg_row) tensor_scalar and DMAd
    to the output slice.
"""

import math

import numpy as np

import concourse.bacc as bacc
import concourse.bass as bass
import concourse.mybir as mybir
import concourse.tile as tile
from concourse.bass_utils import run_bass_kernel_spmd
from concourse.tile import add_dep_helper

P = 128
NCORES = 8
FEAT = 128
EMBED = 128
F32 = mybir.dt.float32
F16 = mybir.dt.float16
I16 = mybir.dt.int16


# ----------------------------------------------------------------------------
# host-side preprocessing (index plumbing only -- no float math on the data
# path except the rsqrt(deg) scale factors, which the device applies)
# ----------------------------------------------------------------------------

def _assign_bins(deg, nbins):
    """Balanced assignment of output rows to bins (<=128 rows per bin).

    Round-based LPT: each round hands the highest-degree unassigned rows to
    the least-loaded bins that still have space.  Returns (bin_of, slot_of).
    """
    n = deg.shape[0]
    order = np.argsort(-deg, kind="stable")
    bin_of = np.empty(n, np.int32)
    slot_of = np.empty(n, np.int32)
    load = np.zeros(nbins, np.float64)
    count = np.zeros(nbins, np.int32)
    pos = 0
    while pos < n:
        avail = np.nonzero(count < P)[0]
        take = min(avail.size, n - pos)
        rows_rd = order[pos : pos + take]
        b = avail[np.argsort(load[avail], kind="stable")[:take]]
        bin_of[rows_rd] = b
        slot_of[rows_rd] = count[b]
        count[b] += 1
        load[b] += deg[rows_rd]
        pos += take
    return bin_of, slot_of


def _prepare(x, W, row, col, cpc, cw, sc):
    """Build all per-core device input arrays.  cpc = chunks (bins) per core.

    The h2 table lives in DRAM in phase-1-chunk-major order: phase-1
    processes nodes in chunks of cw; within chunk k (ntile_k = cw/128 node
    tiles) node j = i*128+p (i = node tile, p = partition) sits at table row
    jp = k*cw + p*ntile_k + (i - k*cw/128) -- so each chunk's h2 write is one
    contiguous run per partition.  Gather indices are int16, so the table is
    split at jp = sc*cw (chunk boundary sc); edges are routed by their half,
    and the 'hi' half [sc*cw, n_pad) is computed FIRST in phase 1 so hi-side
    gathers overlap the rest of phase 1.
    """
    N = x.shape[0]
    E = row.shape[0]
    nbins = NCORES * cpc
    assert nbins * P >= N, (nbins, N)
    n_pad = math.ceil(N / P) * P
    nt = n_pad // P
    cwt = cw // P
    split_jp = min(sc * cw, n_pad)
    assert split_jp <= 2**15, (sc, cw)

    deg = np.bincount(row, minlength=N).astype(np.float64)
    assert deg.min() >= 1
    r = (1.0 / np.sqrt(deg)).astype(np.float32)

    bin_of, slot_of = _assign_bins(deg, nbins)

    # group edges by (destination bin, col table-half)
    eb = bin_of[row].astype(np.int64)
    ed = slot_of[row].astype(np.float32)
    ci = col // P
    ck = ci // cwt
    ntile_k = np.minimum(cwt, nt - ck * cwt)
    jp = ck * cw + (col % P) * ntile_k + (ci - ck * cwt)
    par = (jp >= split_jp).astype(np.int64)
    pidx = np.where(jp >= split_jp, jp - split_jp, jp).astype(np.int16)
    key = eb * 2 + par
    ordk = np.argsort(key, kind="stable")
    counts = np.bincount(key, minlength=nbins * 2)
    nba = math.ceil(int(counts[0::2].max()) / P)
    nbb = math.ceil(int(counts[1::2].max()) / P)
    assert nba + nbb > 0
    cap_a, cap_b = nba * P, nbb * P

    idx_a = np.zeros((nbins, max(cap_a, 1)), np.int16)
    dlt_a = np.full((nbins, max(cap_a, 1)), -1.0, np.float32)
    idx_b = np.zeros((nbins, max(cap_b, 1)), np.int16)
    dlt_b = np.full((nbins, max(cap_b, 1)), -1.0, np.float32)
    starts = np.concatenate([[0], np.cumsum(counts)])
    ks = key[ordk]
    pig = (np.arange(E) - starts[ks]).astype(np.int64)
    m = (ks & 1) == 0
    idx_a[ks[m] >> 1, pig[m]] = pidx[ordk][m]
    dlt_a[ks[m] >> 1, pig[m]] = ed[ordk][m]
    m = ~m
    idx_b[ks[m] >> 1, pig[m]] = pidx[ordk][m]
    dlt_b[ks[m] >> 1, pig[m]] = ed[ordk][m]

    # deltas per bin: [P, nbx] with delta[p, b] = slot of edge b*128+p (or -1)
    d_a = dlt_a[:, :cap_a].reshape(nbins, nba, P).transpose(0, 2, 1)
    d_b = dlt_b[:, :cap_b].reshape(nbins, nbb, P).transpose(0, 2, 1)

    # gather indices per bin: wrapped in 16 partitions, replicated 8x
    def mk_idx(idx, cap):
        t = idx[:, :cap].reshape(nbins, cap // 16, 16).transpose(0, 2, 1)
        return np.tile(t, (1, 8, 1))  # [nbins, 128, cap // 16]

    ia = mk_idx(idx_a, cap_a)
    ib = mk_idx(idx_b, cap_b)

    # per-slot output scale
    rout_bins = np.zeros((nbins, P), np.float32)
    rout_bins[bin_of, slot_of] = r

    # phase-1 arrays
    xT = np.zeros((FEAT, n_pad), np.float32)
    xT[:, :N] = np.ascontiguousarray(x.T)
    r_pad = np.zeros(n_pad, np.float32)
    r_pad[:N] = r
    r_node = np.ascontiguousarray(r_pad.reshape(nt, P).T)
    iota = np.tile(np.arange(P, dtype=np.float16), (P, max(nba, nbb, 1)))

    def pack(arr, width, dt):
        # [nbins, P, width] -> per-core [P, cpc*width]
        out = []
        for dd in range(NCORES):
            b0, b1 = dd * cpc, (dd + 1) * cpc
            out.append(
                np.ascontiguousarray(
                    arr[b0:b1].transpose(1, 0, 2).reshape(P, cpc * width)
                ).astype(dt)
            )
        return out

    da_c = pack(d_a, nba, np.float16)
    db_c = pack(d_b, max(nbb, 0), np.float16) if nbb else None
    ia_c = pack(ia, nba * 8, np.int16)
    ib_c = pack(ib, nbb * 8, np.int16) if nbb else None

    core_maps = []
    for dd in range(NCORES):
        b0, b1 = dd * cpc, (dd + 1) * cpc
        m = {
            "xT": xT,
            "W": np.ascontiguousarray(W.astype(np.float32)),
            "r_node": r_node,
            "iota": iota,
            "delta_a": da_c[dd],
            "idx_a": ia_c[dd],
            "r_out": np.ascontiguousarray(rout_bins[b0:b1].T),
        }
        if nbb:
            m["delta_b"] = db_c[dd]
            m["idx_b"] = ib_c[dd]
        core_maps.append(m)

    gslot = bin_of.astype(np.int64) * P + slot_of.astype(np.int64)
    return core_maps, gslot, nba, nbb, n_pad


# ----------------------------------------------------------------------------
# device kernel
# ----------------------------------------------------------------------------

def _build(n_pad, cpc, nba, nbb, cw, sc):
    nt = n_pad // P
    split_jp = min(sc * cw, n_pad)
    nbmax = max(nba, nbb, 1)

    nc = bacc.Bacc(None, target_bir_lowering=False, debug=False, num_swdge_queues=4)
    xT_d = nc.declare_dram_parameter("xT", [P, n_pad], F32, isOutput=False)
    W_d = nc.declare_dram_parameter("W", [P, EMBED], F32, isOutput=False)
    rn_d = nc.declare_dram_parameter("r_node", [P, nt], F32, isOutput=False)
    io_d = nc.declare_dram_parameter("iota", [P, nbmax * P], F16, isOutput=False)
    da_d = nc.declare_dram_parameter("delta_a", [P, cpc * nba], F16, isOutput=False)
    ia_d = nc.declare_dram_parameter("idx_a", [P, cpc * nba * 8], I16, isOutput=False)
    if nbb:
        db_d = nc.declare_dram_parameter("delta_b", [P, cpc * nbb], F16, isOutput=False)
        ib_d = nc.declare_dram_parameter("idx_b", [P, cpc * nbb * 8], I16, isOutput=False)
    ro_d = nc.declare_dram_parameter("r_out", [P, cpc], F32, isOutput=False)
    out_d = nc.declare_dram_parameter("out", [cpc * P, EMBED], F32, isOutput=True)
    # chunk-major h2 rows (see _prepare docstring)
    h2_d = nc.dram_tensor("h2buf", [P * nt, EMBED], F16)

    starts = list(range(0, n_pad, cw))
    hi_starts = [s for s in starts if s >= split_jp]
    lo_starts = [s for s in starts if s < split_jp]

    with tile.TileContext(nc) as tc:
        with (
            tc.tile_pool(name="const", bufs=1) as constp,
            tc.tile_pool(name="ph1", bufs=4) as ph1,
            tc.tile_pool(name="ps1", bufs=4, space="PSUM") as ps1,
            tc.tile_pool(name="gA", bufs=3) as gAp,
            tc.tile_pool(name="gB", bufs=3) as gBp,
            tc.tile_pool(name="sS", bufs=3) as sSp,
            tc.tile_pool(name="acc", bufs=1) as accp,
            tc.tile_pool(name="ps2", bufs=4, space="PSUM") as ps2,
            tc.tile_pool(name="outp", bufs=4) as outp,
        ):
            W_sb = constp.tile([P, EMBED], F32)
            rn_sb = constp.tile([P, nt], F32)
            io_sb = constp.tile([P, nbmax, P], F16)
            da_sb = constp.tile([P, cpc * nba], F16)
            ia_sb = constp.tile([P, cpc * nba * 8], I16)
            ro_sb = constp.tile([P, cpc], F32)
            nc.sync.dma_start(out=W_sb[:], in_=W_d[:])
            nc.sync.dma_start(out=rn_sb[:], in_=rn_d[:])
            nc.sync.dma_start(
                out=io_sb[:], in_=io_d[:].rearrange("p (a b) -> p a b", b=P)
            )
            nc.sync.dma_start(out=da_sb[:], in_=da_d[:])
            nc.sync.dma_start(out=ia_sb[:], in_=ia_d[:])
            nc.sync.dma_start(out=ro_sb[:], in_=ro_d[:])
            if nbb:
                db_sb = constp.tile([P, cpc * nbb], F16, name="db_sb")
                ib_sb = constp.tile([P, cpc * nbb * 8], I16, name="ib_sb")
                nc.sync.dma_start(out=db_sb[:], in_=db_d[:])
                nc.sync.dma_start(out=ib_sb[:], in_=ib_d[:])

            # accumulators for the two-pass combine: [P, cpc, EMBED] f32
            acc_sb = accp.tile([P, cpc, EMBED], F32, tag="acc", name="acc_sb") if nbb else None

            # ---------------- phase 1: h2 = (x @ W) * rsqrt(deg) ------------
            # hi chunks first: hi-side gathers (pass B) start while the lo
            # half of phase 1 is still running.
            hi_writes, lo_writes = [], []
            for start in hi_starts + lo_starts:
                w = min(cw, n_pad - start)
                ntile = w // P
                xt = ph1.tile([P, cw], F32, tag="xt")
                nc.sync.dma_start(out=xt[:, :w], in_=xT_d[:, start : start + w])
                stage = ph1.tile([P, cw], F16, tag="stage")
                for g0 in range(0, ntile, 4):
                    gn = min(4, ntile - g0)
                    ps = ps1.tile([P, 4, P], F32)
                    for i in range(gn):
                        t = g0 + i
                        nc.tensor.matmul(
                            ps[:, i, :],
                            xt[:, t * P : (t + 1) * P],
                            W_sb[:],
                            start=True,
                            stop=True,
                        )
                    tglob = start // P + g0
                    scl = (
                        rn_sb[:, tglob : tglob + gn]
                        .unsqueeze(2)
                        .to_broadcast([P, gn, P])
                    )
                    st_view = stage[:, g0 * P : (g0 + gn) * P].rearrange(
                        "p (a b) -> p a b", b=P
                    )
                    nc.any.tensor_tensor(
                        out=st_view,
                        in0=ps[:, :gn, :],
                        in1=scl,
                        op=mybir.AluOpType.mult,
                    )
                # contiguous chunk-major write
                wi = nc.sync.dma_start(
                    out=h2_d[start : start + w, :].rearrange(
                        "(p l) f -> p (l f)", l=ntile
                    ),
                    in_=stage[:, :w],
                )
                (hi_writes if start >= split_jp else lo_writes).append(wi.ins)

            bar_hi = nc.gpsimd.nop(nofuse=True, hint="h2_hi_ready")
            for wi in hi_writes:
                add_dep_helper(bar_hi.ins, wi, reason="hi gathers wait on h2 hi")
            bar_lo = nc.gpsimd.nop(nofuse=True, hint="h2_lo_ready")
            for wi in lo_writes:
                add_dep_helper(bar_lo.ins, wi, reason="lo gathers wait on h2 lo")

            # ---------------- phase 2: gather + segment-sum -----------------
            lo_ap = h2_d[0:split_jp, :]
            hi_ap = h2_d[split_jp : P * nt, :] if split_jp < P * nt else None

            def seg_pass(cidx, qn, nbx, tbl_ap, ixs, dls, pool, bar):
                """gather + one-hot + PE accumulate for one (chunk, side)."""
                gt = pool.tile([P, nbx, EMBED], F16, tag="gt", name="gt")
                gi = nc.gpsimd.dma_gather(
                    gt[:],
                    tbl_ap,
                    ixs[:, cidx * nbx * 8 : (cidx + 1) * nbx * 8],
                    nbx * P,
                    nbx * P,
                    EMBED,
                    single_packet=False,
                    queue_num=qn,
                )
                add_dep_helper(gi.ins, bar.ins, reason="h2 half ready")
                S = sSp.tile([P, nbmax, P], F16, tag="S", name="S")
                dsl = (
                    dls[:, cidx * nbx : (cidx + 1) * nbx]
                    .unsqueeze(2)
                    .to_broadcast([P, nbx, P])
                )
                nc.vector.tensor_tensor(
                    out=S[:, :nbx, :],
                    in0=dsl,
                    in1=io_sb[:, :nbx, :],
                    op=mybir.AluOpType.is_equal,
                )
                ps = ps2.tile([P, EMBED], F32, tag="ps2t", name="ps2t")
                for b in range(nbx):
                    nc.tensor.matmul(
                        ps[:],
                        S[:, b, :],
                        gt[:, b, :],
                        start=(b == 0),
                        stop=(b == nbx - 1),
                    )
                return ps

            if nbb:
                # pass B: hi-side edges; starts as soon as h2-hi is written
                for c in range(cpc):
                    ps = seg_pass(c, c % 4, nbb, hi_ap, ib_sb, db_sb, gBp, bar_hi)
                    nc.any.tensor_copy(out=acc_sb[:, c, :], in_=ps[:])

            # pass A: lo-side edges, combine, scale, write out
            for c in range(cpc):
                ps = seg_pass(c, c % 4, nba, lo_ap, ia_sb, da_sb, gAp, bar_lo)
                ot = outp.tile([P, EMBED], F32)
                if nbb:
                    nc.any.tensor_tensor(
                        out=ot[:],
                        in0=ps[:],
                        in1=acc_sb[:, c, :],
                        op=mybir.AluOpType.add,
                    )
                    nc.any.tensor_scalar(
                        out=ot[:],
                        in0=ot[:],
                        scalar1=ro_sb[:, c : c + 1],
                        scalar2=None,
                        op0=mybir.AluOpType.mult,
                    )
                else:
                    nc.any.tensor_scalar(
                        out=ot[:],
                        in0=ps[:],
                        scalar1=ro_sb[:, c : c + 1],
                        scalar2=None,
                        op0=mybir.AluOpType.mult,
                    )
                nc.sync.dma_start(out=out_d[c * P : (c + 1) * P, :], in_=ot[:])

    nc.compile()
    return nc


# ----------------------------------------------------------------------------
# entry point
# ----------------------------------------------------------------------------

def _run(x, W, row, col, cpc=None, cw=2048, sc=16, trace=False):
    x = np.asarray(x, np.float32)
    W = np.asarray(W, np.float32)
    row = np.asarray(row).astype(np.int64)
    col = np.asarray(col).astype(np.int64)
    N = x.shape[0]
    if cpc is None:
        cpc = math.ceil(N / (NCORES * P))
    cw = min(cw, math.ceil(N / P) * P)
    core_maps, gslot, nba, nbb, n_pad = _prepare(x, W, row, col, cpc, cw, sc)
    nc = _build(n_pad, cpc, nba, nbb, cw, sc)
    res = run_bass_kernel_spmd(
        nc, core_maps, list(range(NCORES)), trace=trace
    )
    big = np.concatenate([res.results[d]["out"] for d in range(NCORES)], axis=0)
    out = np.ascontiguousarray(big[gslot], dtype=np.float32)
    return out, res


def kernel(**inputs):
    out, _ = _run(inputs["x"], inputs["W"], inputs["row"], inputs["col"])
    return out


# revision 40
# speedup vs baseline: 1.3446x; 1.3446x over previous
"""GCN message-passing (GCNConv) on 8 Trainium2 NeuronCores via Bass/Tile.

Math (reference):
    deg[i] = |{e : row[e] == i}|
    h      = x @ W
    out[i] = sum_{e: row[e]==i} h[col[e]] / sqrt(deg[row[e]] * deg[col[e]])

Because the GCN norm is separable (1/sqrt(deg_i) * 1/sqrt(deg_j)), we compute
    h2   = (x @ W) * rsqrt(deg)[:, None]          (phase 1, on device)
    out  = rsqrt(deg)[:, None] * scatter_add(h2[col], row)   (phase 2)
so the per-edge work is a pure gather + segment-sum with NO per-edge flops.

Sharding: output rows are bin-packed into 8*CPC bins of <=128 rows, balanced
by edge count (the "1D row partition" of the hint, load-balanced).  Each core
owns CPC bins.  Every core computes the full h2 table (replicated XW -- it is
tiny next to the gather traffic) into its own HBM, then per bin:
  - dma_gather (GPSIMD SWDGE) pulls the bin's neighbor rows h2[col] into SBUF,
    128 edges per partition-block.  int16 gather indices only address 32k rows,
    so edges are split by col < 32768 and gathered with two table base offsets.
  - a one-hot "segment matrix" S (S[e, slot] = 1 iff edge e belongs to output
    slot) is built on the vector engine with a broadcast is_equal, and the
    segment-sum runs as PE matmuls accumulating into PSUM: out = S.T @ msgs.
  - PSUM is evacuated through a fused *rsqrt(deg_row) tensor_scalar and DMAd
    to the output slice.
"""

import math

import numpy as np

import concourse.bacc as bacc
import concourse.bass as bass
import concourse.mybir as mybir
import concourse.tile as tile
from concourse.bass_utils import run_bass_kernel_spmd
from concourse.tile import add_dep_helper

P = 128
NCORES = 8
FEAT = 128
EMBED = 128
F32 = mybir.dt.float32
F16 = mybir.dt.float16
I16 = mybir.dt.int16


# ----------------------------------------------------------------------------
# host-side preprocessing (index plumbing only -- no float math on the data
# path except the rsqrt(deg) scale factors, which the device applies)
# ----------------------------------------------------------------------------

def _assign_bins(deg, nbins):
    """Balanced assignment of output rows to bins (<=128 rows per bin).

    Round-based LPT: each round hands the highest-degree unassigned rows to
    the least-loaded bins that still have space.  Returns (bin_of, slot_of).
    """
    n = deg.shape[0]
    order = np.argsort(-deg, kind="stable")
    bin_of = np.empty(n, np.int32)
    slot_of = np.empty(n, np.int32)
    load = np.zeros(nbins, np.float64)
    count = np.zeros(nbins, np.int32)
    pos = 0
    while pos < n:
        avail = np.nonzero(count < P)[0]
        take = min(avail.size, n - pos)
        rows_rd = order[pos : pos + take]
        b = avail[np.argsort(load[avail], kind="stable")[:take]]
        bin_of[rows_rd] = b
        slot_of[rows_rd] = count[b]
        count[b] += 1
        load[b] += deg[rows_rd]
        pos += take
    return bin_of, slot_of


def _prepare(x, W, row, col, cpc, cw, sc):
    """Build all per-core device input arrays.  cpc = chunks (bins) per core.

    The h2 table lives in DRAM in phase-1-chunk-major order: phase-1
    processes nodes in chunks of cw; within chunk k (ntile_k = cw/128 node
    tiles) node j = i*128+p (i = node tile, p = partition) sits at table row
    jp = k*cw + p*ntile_k + (i - k*cw/128) -- so each chunk's h2 write is one
    contiguous run per partition.  Gather indices are int16, so the table is
    split at jp = sc*cw (chunk boundary sc); edges are routed by their half,
    and the 'hi' half [sc*cw, n_pad) is computed FIRST in phase 1 so hi-side
    gathers overlap the rest of phase 1.
    """
    N = x.shape[0]
    E = row.shape[0]
    nbins = NCORES * cpc
    assert nbins * P >= N, (nbins, N)
    n_pad = math.ceil(N / P) * P
    nt = n_pad // P
    cwt = cw // P
    split_jp = min(sc * cw, n_pad)
    assert split_jp <= 2**15, (sc, cw)

    deg = np.bincount(row, minlength=N).astype(np.float64)
    assert deg.min() >= 1
    r = (1.0 / np.sqrt(deg)).astype(np.float32)

    bin_of, slot_of = _assign_bins(deg, nbins)

    # group edges by (destination bin, col table-half)
    eb = bin_of[row].astype(np.int64)
    ed = slot_of[row].astype(np.float32)
    ci = col // P
    ck = ci // cwt
    ntile_k = np.minimum(cwt, nt - ck * cwt)
    jp = ck * cw + (col % P) * ntile_k + (ci - ck * cwt)
    par = (jp >= split_jp).astype(np.int64)
    pidx = np.where(jp >= split_jp, jp - split_jp, jp).astype(np.int16)
    key = eb * 2 + par
    ordk = np.argsort(key, kind="stable")
    counts = np.bincount(key, minlength=nbins * 2)
    nba = math.ceil(int(counts[0::2].max()) / P)
    nbb = math.ceil(int(counts[1::2].max()) / P)
    assert nba + nbb > 0
    cap_a, cap_b = nba * P, nbb * P

    idx_a = np.zeros((nbins, max(cap_a, 1)), np.int16)
    dlt_a = np.full((nbins, max(cap_a, 1)), -1.0, np.float32)
    idx_b = np.zeros((nbins, max(cap_b, 1)), np.int16)
    dlt_b = np.full((nbins, max(cap_b, 1)), -1.0, np.float32)
    starts = np.concatenate([[0], np.cumsum(counts)])
    ks = key[ordk]
    pig = (np.arange(E) - starts[ks]).astype(np.int64)
    m = (ks & 1) == 0
    idx_a[ks[m] >> 1, pig[m]] = pidx[ordk][m]
    dlt_a[ks[m] >> 1, pig[m]] = ed[ordk][m]
    m = ~m
    idx_b[ks[m] >> 1, pig[m]] = pidx[ordk][m]
    dlt_b[ks[m] >> 1, pig[m]] = ed[ordk][m]

    nb = nba + nbb
    # deltas per bin: [P, nb] with delta[p, b] = slot of edge b*128+p (or -1)
    d_a = dlt_a[:, :cap_a].reshape(nbins, nba, P).transpose(0, 2, 1)
    d_b = dlt_b[:, :cap_b].reshape(nbins, nbb, P).transpose(0, 2, 1)
    dall = np.concatenate([d_a, d_b], axis=2)  # [nbins, P, nb]

    # gather indices per bin: wrapped in 16 partitions, replicated 8x
    def mk_idx(idx, cap):
        t = idx[:, :cap].reshape(nbins, cap // 16, 16).transpose(0, 2, 1)
        return np.tile(t, (1, 8, 1))  # [nbins, 128, cap // 16]

    idx_all = np.concatenate([mk_idx(idx_a, cap_a), mk_idx(idx_b, cap_b)], axis=2)

    # per-slot output scale
    rout_bins = np.zeros((nbins, P), np.float32)
    rout_bins[bin_of, slot_of] = r

    # phase-1 arrays
    xT = np.zeros((FEAT, n_pad), np.float32)
    xT[:, :N] = np.ascontiguousarray(x.T)
    r_pad = np.zeros(n_pad, np.float32)
    r_pad[:N] = r
    r_node = np.ascontiguousarray(r_pad.reshape(nt, P).T)
    iota = np.tile(np.arange(P, dtype=np.float16), (P, nb))

    idxc = nb * 8  # idx columns per chunk
    core_maps = []
    for dd in range(NCORES):
        b0, b1 = dd * cpc, (dd + 1) * cpc
        core_maps.append(
            {
                "xT": xT,
                "W": np.ascontiguousarray(W.astype(np.float32)),
                "r_node": r_node,
                "iota": iota,
                "delta": np.ascontiguousarray(
                    dall[b0:b1].transpose(1, 0, 2).reshape(P, cpc * nb)
                ).astype(np.float16),
                "idx": np.ascontiguousarray(
                    idx_all[b0:b1].transpose(1, 0, 2).reshape(P, cpc * idxc)
                ),
                "r_out": np.ascontiguousarray(rout_bins[b0:b1].T),
            }
        )

    gslot = bin_of.astype(np.int64) * P + slot_of.astype(np.int64)
    return core_maps, gslot, nba, nbb, n_pad


# ----------------------------------------------------------------------------
# device kernel
# ----------------------------------------------------------------------------

def _build(n_pad, cpc, nba, nbb, cw, sc):
    nt = n_pad // P
    split_jp = min(sc * cw, n_pad)
    nb = nba + nbb
    idxc = nb * 8

    nc = bacc.Bacc(None, target_bir_lowering=False, debug=False, num_swdge_queues=4)
    xT_d = nc.declare_dram_parameter("xT", [P, n_pad], F32, isOutput=False)
    W_d = nc.declare_dram_parameter("W", [P, EMBED], F32, isOutput=False)
    rn_d = nc.declare_dram_parameter("r_node", [P, nt], F32, isOutput=False)
    io_d = nc.declare_dram_parameter("iota", [P, nb * P], F16, isOutput=False)
    dl_d = nc.declare_dram_parameter("delta", [P, cpc * nb], F16, isOutput=False)
    ix_d = nc.declare_dram_parameter("idx", [P, cpc * idxc], I16, isOutput=False)
    ro_d = nc.declare_dram_parameter("r_out", [P, cpc], F32, isOutput=False)
    out_d = nc.declare_dram_parameter("out", [cpc * P, EMBED], F32, isOutput=True)
    # chunk-major h2 rows (see _prepare docstring)
    h2_d = nc.dram_tensor("h2buf", [P * nt, EMBED], F16)

    starts = list(range(0, n_pad, cw))
    hi_starts = [s for s in starts if s >= split_jp]
    lo_starts = [s for s in starts if s < split_jp]

    with tile.TileContext(nc) as tc:
        with (
            tc.tile_pool(name="const", bufs=1) as constp,
            tc.tile_pool(name="ph1", bufs=4) as ph1,
            tc.tile_pool(name="ps1", bufs=4, space="PSUM") as ps1,
            tc.tile_pool(name="gA", bufs=3) as gAp,
            tc.tile_pool(name="gB", bufs=8) as gBp,
            tc.tile_pool(name="sS", bufs=3) as sSp,
            tc.tile_pool(name="ps2", bufs=4, space="PSUM") as ps2,
            tc.tile_pool(name="outp", bufs=4) as outp,
        ):
            W_sb = constp.tile([P, EMBED], F32)
            rn_sb = constp.tile([P, nt], F32)
            io_sb = constp.tile([P, nb, P], F16)
            dl_sb = constp.tile([P, cpc * nb], F16)
            ix_sb = constp.tile([P, cpc * idxc], I16)
            ro_sb = constp.tile([P, cpc], F32)
            nc.sync.dma_start(out=W_sb[:], in_=W_d[:])
            nc.sync.dma_start(out=rn_sb[:], in_=rn_d[:])
            nc.sync.dma_start(
                out=io_sb[:], in_=io_d[:].rearrange("p (a b) -> p a b", b=P)
            )
            nc.sync.dma_start(out=dl_sb[:], in_=dl_d[:])
            nc.sync.dma_start(out=ix_sb[:], in_=ix_d[:])
            nc.sync.dma_start(out=ro_sb[:], in_=ro_d[:])

            # ---------------- phase 1: h2 = (x @ W) * rsqrt(deg) ------------
            # hi chunks first: hi-side gathers (pass B) start while the lo
            # half of phase 1 is still running.
            hi_writes, lo_writes = [], []
            for start in hi_starts + lo_starts:
                w = min(cw, n_pad - start)
                ntile = w // P
                xt = ph1.tile([P, cw], F32, tag="xt")
                nc.sync.dma_start(out=xt[:, :w], in_=xT_d[:, start : start + w])
                stage = ph1.tile([P, cw], F16, tag="stage")
                for g0 in range(0, ntile, 4):
                    gn = min(4, ntile - g0)
                    ps = ps1.tile([P, 4, P], F32)
                    for i in range(gn):
                        t = g0 + i
                        nc.tensor.matmul(
                            ps[:, i, :],
                            xt[:, t * P : (t + 1) * P],
                            W_sb[:],
                            start=True,
                            stop=True,
                        )
                    tglob = start // P + g0
                    scl = (
                        rn_sb[:, tglob : tglob + gn]
                        .unsqueeze(2)
                        .to_broadcast([P, gn, P])
                    )
                    st_view = stage[:, g0 * P : (g0 + gn) * P].rearrange(
                        "p (a b) -> p a b", b=P
                    )
                    nc.any.tensor_tensor(
                        out=st_view,
                        in0=ps[:, :gn, :],
                        in1=scl,
                        op=mybir.AluOpType.mult,
                    )
                # contiguous chunk-major write
                wi = nc.sync.dma_start(
                    out=h2_d[start : start + w, :].rearrange(
                        "(p l) f -> p (l f)", l=ntile
                    ),
                    in_=stage[:, :w],
                )
                (hi_writes if start >= split_jp else lo_writes).append(wi.ins)

            bar_hi = nc.gpsimd.nop(nofuse=True, hint="h2_hi_ready")
            for wi in hi_writes:
                add_dep_helper(bar_hi.ins, wi, reason="hi gathers wait on h2 hi")
            bar_lo = nc.gpsimd.nop(nofuse=True, hint="h2_lo_ready")
            for wi in lo_writes:
                add_dep_helper(bar_lo.ins, wi, reason="lo gathers wait on h2 lo")

            # ---------------- phase 2: gather + segment-sum -----------------
            # Interleaved A/B gather issue (A on queues {0,2}, B on {1,3}) --
            # this pairing measures ~2.7ns/idx vs ~4.4 for uniform streams.
            # B gathers are issued PREFETCH_B chunks ahead so they run while
            # the lo half of phase 1 is still computing.
            lo_ap = h2_d[0:split_jp, :]
            hi_ap = h2_d[split_jp : P * nt, :] if split_jp < P * nt else None
            PREFETCH_B = 6

            def issue_b(c):
                if nbb == 0:
                    return None
                gb = gBp.tile([P, nbb, EMBED], F16, tag="gB", name="gb")
                g2 = nc.gpsimd.dma_gather(
                    gb[:],
                    hi_ap,
                    ix_sb[:, c * idxc + nba * 8 : (c + 1) * idxc],
                    nbb * P,
                    nbb * P,
                    EMBED,
                    single_packet=False,
                    queue_num=(2 * c + 1) % 4,
                )
                add_dep_helper(g2.ins, bar_hi.ins, reason="h2 hi ready")
                return gb

            gB_tiles = {}
            for c in range(min(PREFETCH_B, cpc)):
                gB_tiles[c] = issue_b(c)

            for c in range(cpc):
                ga = None
                if nba > 0:
                    ga = gAp.tile([P, nba, EMBED], F16, tag="gA", name="ga")
                    g1 = nc.gpsimd.dma_gather(
                        ga[:],
                        lo_ap,
                        ix_sb[:, c * idxc : c * idxc + nba * 8],
                        nba * P,
                        nba * P,
                        EMBED,
                        single_packet=False,
                        queue_num=(2 * c) % 4,
                    )
                    add_dep_helper(g1.ins, bar_lo.ins, reason="h2 lo ready")
                if c + PREFETCH_B < cpc:
                    gB_tiles[c + PREFETCH_B] = issue_b(c + PREFETCH_B)
                gb = gB_tiles.pop(c, None)

                S = sSp.tile([P, nb, P], F16, tag="S", name="S")
                dsl = (
                    dl_sb[:, c * nb : (c + 1) * nb]
                    .unsqueeze(2)
                    .to_broadcast([P, nb, P])
                )
                nc.vector.tensor_tensor(
                    out=S[:], in0=dsl, in1=io_sb[:], op=mybir.AluOpType.is_equal
                )
                ps = ps2.tile([P, EMBED], F32, tag="ps2t", name="ps2t")
                for b in range(nb):
                    rhs = ga[:, b, :] if b < nba else gb[:, b - nba, :]
                    nc.tensor.matmul(
                        ps[:],
                        S[:, b, :],
                        rhs,
                        start=(b == 0),
                        stop=(b == nb - 1),
                    )
                ot = outp.tile([P, EMBED], F32)
                nc.any.tensor_scalar(
                    out=ot[:],
                    in0=ps[:],
                    scalar1=ro_sb[:, c : c + 1],
                    scalar2=None,
                    op0=mybir.AluOpType.mult,
                )
                nc.sync.dma_start(out=out_d[c * P : (c + 1) * P, :], in_=ot[:])

    nc.compile()
    return nc


# ----------------------------------------------------------------------------
# entry point
# ----------------------------------------------------------------------------

def _run(x, W, row, col, cpc=None, cw=2048, sc=16, trace=False):
    x = np.asarray(x, np.float32)
    W = np.asarray(W, np.float32)
    row = np.asarray(row).astype(np.int64)
    col = np.asarray(col).astype(np.int64)
    N = x.shape[0]
    if cpc is None:
        cpc = math.ceil(N / (NCORES * P))
    cw = min(cw, math.ceil(N / P) * P)
    core_maps, gslot, nba, nbb, n_pad = _prepare(x, W, row, col, cpc, cw, sc)
    nc = _build(n_pad, cpc, nba, nbb, cw, sc)
    res = run_bass_kernel_spmd(
        nc, core_maps, list(range(NCORES)), trace=trace
    )
    big = np.concatenate([res.results[d]["out"] for d in range(NCORES)], axis=0)
    out = np.ascontiguousarray(big[gslot], dtype=np.float32)
    return out, res


def kernel(**inputs):
    out, _ = _run(inputs["x"], inputs["W"], inputs["row"], inputs["col"])
    return out
